# revision 26
# baseline (speedup 1.0000x reference)
"""Nearest-neighbor retrieval kernel for Trainium2 (8 NeuronCores, SPMD).

Problem: dis[i] = mean((in_vel - train_obs_vel[i])**2); return
train_target_vel[argmin(dis)].

Strategy: only train_obs_vel has to stream through the devices. The device
pass is a bf16 *screen*: it computes approximate keys
k_i ~= sum((x_i - q)^2) for every sample (bf16 halves HBM traffic to
~26.5 MB/core and doubles VectorE throughput). The host then recomputes
exact f32 keys for the top-1024 screened candidates (~1 M flops) and picks
the true argmin — bf16 key noise is ~+-0.3 on a min-gap of ~4, so the true
argmin is inside the top-1024 with overwhelming margin, and the final
result is bit-exact.

Sharding: 12500 rows per core, padded to 12544 = 128*98 so DMA tiles have
128 partitions (the HWDGE only engages all 16 SDMA engines for
128-partition descriptor lists; at 125 partitions it uses 5 and runs 3x
slower). Per column (sample-slice) the engines split work:
  - VectorE  TENSOR_TENSOR(subtract) bf16 (2x mode)   -> diff
  - ScalarE  ACTIVATE(Square, accum_out)              -> key (most cols)
  - VectorE  TENSOR_TENSOR(mult) + TENSOR_REDUCE(add) -> key (offload cols)
so ScalarE (dtype-independent 1 elem/cycle) stops being the bottleneck.
The tiny [128, 98] f32 key tile per core returns to the host; no device
collectives are needed.
"""

import sys

sys.path.insert(0, "/opt/trn_rl_repo")

import ml_dtypes
import numpy as np

import concourse.bacc as bacc
import concourse.mybir as mybir
import concourse.tile as tile
from concourse.bass_utils import run_bass_kernel_spmd

# Problem shapes (hardcoded per harness contract)
N = 100000
T_OBS = 16
T_OUT = 25
D = 66
F = T_OBS * D  # 1056 features per sample
CORES = 8
PER = N // CORES  # 12500 samples per core
P = 128  # SBUF partitions
C = 98  # samples (columns) per partition
PER_PAD = P * C  # 12544 padded samples per core
PAD_VAL = 1.0e4  # pad rows get a huge distance; never the argmin
S = 2  # samples per partition per DMA tile
N_VCOLS = 19  # columns whose square+reduce runs on VectorE (mul+reduce)
N_GSUBS = 0  # GpSimd subtract was a net loss (port contention)
TOPK = 1024  # host-side exact recheck pool


def _spread(n, total=C):
    """n column indices spread evenly over [0, total)."""
    return {int(round(i * total / n)) % total for i in range(n)}

_f32 = mybir.dt.float32
_bf16 = mybir.dt.bfloat16
_bf16_np = ml_dtypes.bfloat16


def build_nc(s=S, xin_bufs=12, n_vcols=N_VCOLS, n_gsubs=N_GSUBS):
    ntiles = C // s
    assert ntiles * s == C
    vcols = _spread(n_vcols)
    gsubs = _spread(n_gsubs)
    nc = bacc.Bacc("TRN2", target_bir_lowering=False, debug=False)
    x = nc.dram_tensor("x", [PER_PAD, F], _bf16, kind="ExternalInput")
    qb = nc.dram_tensor("qb", [P, F], _bf16, kind="ExternalInput")
    key_out = nc.dram_tensor("key", [P, C], _f32, kind="ExternalOutput")

    # [12544, 1056] -> [128 partitions, 98*1056 contiguous bf16]
    xr = x[:].rearrange("(p c) d -> p (c d)", p=P)

    with tile.TileContext(nc) as tc:
        with (
            tc.tile_pool(name="xin", bufs=xin_bufs) as xpool,
            tc.tile_pool(name="qpool", bufs=1) as qpool,
            tc.tile_pool(name="scratch", bufs=6) as spool,
            tc.tile_pool(name="acc", bufs=1) as apool,
        ):
            q_tile = qpool.tile([P, F], _bf16)
            nc.sync.dma_start(out=q_tile[:], in_=qb[:])

            key_t = apool.tile([P, C], _f32)

            for t in range(ntiles):
                xt = xpool.tile([P, s * F], _bf16, tag="xt")
                nc.sync.dma_start(
                    out=xt[:], in_=xr[:, t * s * F : (t + 1) * s * F]
                )
                for j in range(s):
                    col = t * s + j
                    xs = xt[:, j * F : (j + 1) * F]
                    diff = spool.tile([P, F], _bf16, tag="diff")
                    sub_eng = nc.gpsimd if col in gsubs else nc.vector
                    sub_eng.tensor_sub(diff[:], xs, q_tile[:])
                    kcol = key_t[:, col : col + 1]
                    if col in vcols:
                        # VectorE path: mult + reduce
                        sq = spool.tile([P, F], _bf16, tag="vsq")
                        nc.vector.tensor_mul(sq[:], diff[:], diff[:])
                        nc.vector.tensor_reduce(
                            kcol,
                            sq[:],
                            axis=mybir.AxisListType.X,
                            op=mybir.AluOpType.add,
                        )
                    else:
                        # ScalarE path: Square with free-axis accumulate
                        sq = spool.tile([P, F], _bf16, tag="ssq")
                        nc.scalar.activation(
                            out=sq[:],
                            in_=diff[:],
                            func=mybir.ActivationFunctionType.Square,
                            accum_out=kcol,
                        )

            nc.sync.dma_start(out=key_out[:], in_=key_t[:])
    nc.compile()
    return nc


_nc_cache = {}


def _get_nc():
    key = (S, N_VCOLS, N_GSUBS)
    if key not in _nc_cache:
        _nc_cache[key] = build_nc()
    return _nc_cache[key]


def make_in_maps(in_vel, train_obs_vel):
    q = np.asarray(in_vel, dtype=np.float32).reshape(F)
    qbn = np.ascontiguousarray(
        np.broadcast_to(q.astype(_bf16_np), (P, F))
    )
    X = np.asarray(train_obs_vel, dtype=np.float32).reshape(N, F)
    Xb = X.astype(_bf16_np)
    in_maps = []
    for c in range(CORES):
        xp = np.full((PER_PAD, F), PAD_VAL, dtype=_bf16_np)
        xp[:PER] = Xb[c * PER : (c + 1) * PER]
        in_maps.append({"x": xp, "qb": qbn})
    return in_maps


def finish(results, in_vel, train_obs_vel, train_target_vel):
    # keys[core][p, col] screens padded-local sample p*C + col; flattening
    # in C order reproduces the padded-local sample order.
    keys = np.stack([np.asarray(r["key"]) for r in results])  # [8, P, C]
    flat = keys.reshape(CORES, PER_PAD)[:, :PER].reshape(-1)  # drop pads
    k = min(TOPK, flat.size)
    cand = np.sort(np.argpartition(flat, k - 1)[:k])
    # exact f32 recheck of the screened candidates
    q = np.asarray(in_vel, dtype=np.float32).reshape(F)
    X = np.asarray(train_obs_vel, dtype=np.float32).reshape(N, F)
    d = X[cand] - q
    exact = np.einsum("ij,ij->i", d, d)
    best = int(cand[int(exact.argmin())])
    out = np.asarray(train_target_vel)[best]
    return np.ascontiguousarray(out)


def kernel(in_vel, train_obs_vel, train_target_vel):
    nc = _get_nc()
    in_maps = make_in_maps(in_vel, train_obs_vel)
    res = run_bass_kernel_spmd(nc, in_maps, list(range(CORES)))
    return finish(res.results, in_vel, train_obs_vel, train_target_vel)
